# revision 12
# baseline (speedup 1.0000x reference)
"""Multi-head attention + residual + LayerNorm on 8 TRN2 NeuronCores.

Problem: nn_MultiHeadAttention (B=2, S=2048, D=1024, H=16, dh=64).
Returns (output [2,2048,1024], attn [32,2048,2048]) matching the jax reference.

Sharding: core c handles batch b=c//4 and query slice qs=c%4 (512 queries),
all 16 heads. No collectives. Each core:
  1. Projects K^T [dh,tok], V [tok,dh] (full 2048 keys of its batch) and
     Q^T [dh,q] (its 512 queries) in fp32r (tf32-like, full PE rate).
  2. Pass A per (head, 128-query block): scores S[q,kk] via PE, exp on ACT
     with accum_out row sums, 1/sum on DVE, normalize on GPSIMD, DMA out
     the attention probabilities.
  3. Pass B per head: scores recomputed transposed S^T[kk,q], exp'd, and
     fed as the moving operand of the context matmul C^T[dh,q] accumulated
     in PSUM. Row normalization (per head) is applied during the PSUM
     evacuation using a [p,512] broadcast of 1/rowsum built by a small DMA
     round trip through DRAM.
  4. Output projection from C^T, + residual + (bo + Wo@bv) + LayerNorm.

Host-side prep: activations and weights are passed pre-transposed; the
attention scale 1/8 is folded into WqT/bq; bv is folded into badd=bo+Wo@bv
(softmax rows sum to 1 exactly in math, ~1e-7 in practice).
"""

import sys
from contextlib import ExitStack

for _p in ("/opt/trn_rl_repo", "/root/.axon_site/_ro/trn_rl_repo"):
    if _p not in sys.path:
        sys.path.insert(0, _p)

import numpy as np

import concourse.bass as bass
import concourse.tile as tile
from concourse import bacc, mybir
from concourse.bass_utils import run_bass_kernel_spmd

F32 = mybir.dt.float32
F32R = mybir.dt.float32r
AF = mybir.ActivationFunctionType
OP = mybir.AluOpType

B, S, D, H, DH = 2, 2048, 1024, 16, 64
SQ = 512  # queries per core
NCORES = 8
LN_EPS = 1e-5

PROFILE = False
TRACE_DIR = None
LAST_EXEC_NS = None
_NC_CACHE = None


def _bcast(ap, p):
    """Broadcast an AP across p partitions (partition step 0)."""
    return bass.AP(tensor=ap.tensor, offset=ap.offset, ap=[[0, p], *ap.ap])


def build():
    nc = bacc.Bacc()
    qT_d = nc.declare_dram_parameter("queryT", [D, SQ], F32R, isOutput=False)
    qN_d = nc.declare_dram_parameter("query_nat", [SQ, D], F32, isOutput=False)
    kT_d = nc.declare_dram_parameter("keyT", [D, S], F32R, isOutput=False)
    vT_d = nc.declare_dram_parameter("valueT", [D, S], F32R, isOutput=False)
    wqT_d = nc.declare_dram_parameter("wqT", [D, D], F32R, isOutput=False)
    wkT_d = nc.declare_dram_parameter("wkT", [D, D], F32R, isOutput=False)
    wvT_d = nc.declare_dram_parameter("wvT", [D, D], F32R, isOutput=False)
    woT_d = nc.declare_dram_parameter("woT", [D, D], F32R, isOutput=False)
    bq_d = nc.declare_dram_parameter("bq", [D], F32, isOutput=False)
    bk_d = nc.declare_dram_parameter("bk", [D], F32, isOutput=False)
    badd_d = nc.declare_dram_parameter("badd", [D], F32, isOutput=False)
    gam_d = nc.declare_dram_parameter("gamma", [D], F32, isOutput=False)
    bet_d = nc.declare_dram_parameter("beta", [D], F32, isOutput=False)
    out_d = nc.declare_dram_parameter("out", [SQ, D], F32, isOutput=True)
    attn_d = nc.declare_dram_parameter("attn", [H, SQ, S], F32, isOutput=True)

    kT_r = kT_d.rearrange("(kt p) t -> p kt t", p=128)
    vT_r = vT_d.rearrange("(kt p) t -> p kt t", p=128)
    qT_r = qT_d.rearrange("(kt p) t -> p kt t", p=128)

    with tile.TileContext(nc) as tc, ExitStack() as rstack:
        small = rstack.enter_context(
            tc.tile_pool(name="small", bufs=8, side="right")
        )
        with tc.tile_pool(name="dram", bufs=1, space="DRAM") as dpool:
            CT = None  # allocated after the projections (SBUF pressure)
            rr_t = dpool.tile([H, SQ], F32, tag="rr")

            with tc.tile_pool(name="persist", bufs=1) as pp:
                Vn = pp.tile([128, 16, D], F32R, tag="V")  # [tok, dh] 8MB
                KT = pp.tile([128, 8, S], F32R, tag="KT")  # [dh, tok] 8MB
                QT = pp.tile([128, 8, SQ], F32R, tag="QT")  # [dh, q] 2MB

                bk_sb = small.tile([128, 8], F32, tag="bk")
                nc.sync.dma_start(
                    out=bk_sb, in_=bk_d.rearrange("(mt p) -> p mt", p=128)
                )
                bq_sb = small.tile([128, 8], F32, tag="bq")
                nc.sync.dma_start(
                    out=bq_sb, in_=bq_d.rearrange("(mt p) -> p mt", p=128)
                )

                # ---- V' = value @ Wv.T (bias folded into badd) ----
                with (
                    tc.tile_pool(name="wv", bufs=1) as wvp,
                    tc.tile_pool(name="vstream", bufs=2) as vsp,
                    tc.tile_pool(name="psP", bufs=4, space="PSUM") as psP,
                ):
                    WvT = wvp.tile([128, 8, D], F32R, tag="WvT")
                    nc.sync.dma_start(
                        out=WvT, in_=wvT_d.rearrange("(kt p) o -> p kt o", p=128)
                    )
                    for tt in range(16):
                        vch = vsp.tile([128, 8, 128], F32R, tag="vch")
                        nc.sync.dma_start(
                            out=vch, in_=vT_r[:, :, tt * 128 : (tt + 1) * 128]
                        )
                        for nh in range(2):
                            ps = psP.tile([128, 512], F32, tag="pp")
                            for kt in range(8):
                                nc.tensor.matmul(
                                    ps,
                                    vch[:, kt, :],
                                    WvT[:, kt, nh * 512 : (nh + 1) * 512],
                                    start=(kt == 0),
                                    stop=(kt == 7),
                                )
                            nc.vector.tensor_copy(
                                out=Vn[:, tt, nh * 512 : (nh + 1) * 512], in_=ps
                            )

                # ---- K'^T = Wk @ key^T + bk ----
                with (
                    tc.tile_pool(name="wk", bufs=1) as wkp,
                    tc.tile_pool(name="kstream", bufs=1) as ksp,
                    tc.tile_pool(name="psP2", bufs=4, space="PSUM") as psP2,
                ):
                    WkT = wkp.tile([128, 8, D], F32R, tag="WkT")
                    nc.sync.dma_start(
                        out=WkT, in_=wkT_d.rearrange("(kt p) o -> p kt o", p=128)
                    )
                    for nt in range(8):  # 256-token chunks
                        kch = ksp.tile([128, 8, 256], F32R, tag="kch")
                        nc.sync.dma_start(
                            out=kch, in_=kT_r[:, :, nt * 256 : (nt + 1) * 256]
                        )
                        for mt in range(8):
                            ps = psP2.tile([128, 256], F32, tag="pp2")
                            for kt in range(8):
                                nc.tensor.matmul(
                                    ps,
                                    WkT[:, kt, mt * 128 : (mt + 1) * 128],
                                    kch[:, kt, :],
                                    start=(kt == 0),
                                    stop=(kt == 7),
                                )
                            nc.vector.tensor_scalar_add(
                                out=KT[:, mt, nt * 256 : (nt + 1) * 256],
                                in0=ps,
                                scalar1=bk_sb[:, mt : mt + 1],
                            )

                # ---- Q'^T = Wq_scaled @ query^T + bq_scaled ----
                with (
                    tc.tile_pool(name="wq", bufs=2) as wqp,
                    tc.tile_pool(name="qstream", bufs=1) as qsp,
                    tc.tile_pool(name="psP3", bufs=4, space="PSUM") as psP3,
                ):
                    wqT_r = wqT_d.rearrange("(kt p) o -> p kt o", p=128)
                    qTs = qsp.tile([128, 8, SQ], F32R, tag="qTs")
                    nc.sync.dma_start(out=qTs, in_=qT_r)
                    for mt in range(8):
                        wqch = wqp.tile([128, 8, 128], F32R, tag="wqch")
                        nc.sync.dma_start(
                            out=wqch, in_=wqT_r[:, :, mt * 128 : (mt + 1) * 128]
                        )
                        ps = psP3.tile([128, 512], F32, tag="pp3")
                        for kt in range(8):
                            nc.tensor.matmul(
                                ps,
                                wqch[:, kt, :],
                                qTs[:, kt, :],
                                start=(kt == 0),
                                stop=(kt == 7),
                            )
                        nc.vector.tensor_scalar_add(
                            out=QT[:, mt, :], in0=ps, scalar1=bq_sb[:, mt : mt + 1]
                        )

                # context^T stored per head on partitions 0-63: [dh%64, head, q]
                keep = rstack.enter_context(
                    tc.tile_pool(name="keep", bufs=1, side="right")
                )
                CT = keep.tile([64, H, SQ], F32R, tag="CT")

                # ---- attention heads ----
                with (
                    tc.tile_pool(name="epool", bufs=2) as epool,
                    tc.tile_pool(name="etpool", bufs=3) as etpool,
                    tc.tile_pool(name="rrbp", bufs=2) as rrbp,
                    tc.tile_pool(name="psA", bufs=2, space="PSUM") as psA,
                    tc.tile_pool(name="psB", bufs=2, space="PSUM") as psB,
                    tc.tile_pool(name="psC", bufs=2, space="PSUM") as psC,
                ):
                    for pair in range(8):
                        heads = (2 * pair, 2 * pair + 1)
                        # pass A: probabilities in [q, kk] + row sums
                        for h in heads:
                            pb, mt = (h % 2) * 64, h // 2
                            for qb in range(4):
                                e_t = epool.tile([128, S], F32, tag="E")
                                racc = small.tile([128, 2], F32, tag="racc")
                                for half in range(2):
                                    ps = psA.tile([128, 1024], F32, tag="sA")
                                    for nt in range(2):
                                        j = half * 2 + nt
                                        nc.tensor.matmul(
                                            ps[:, nt * 512 : (nt + 1) * 512],
                                            QT[
                                                pb : pb + 64,
                                                mt,
                                                qb * 128 : (qb + 1) * 128,
                                            ],
                                            KT[
                                                pb : pb + 64,
                                                mt,
                                                j * 512 : (j + 1) * 512,
                                            ],
                                            start=True,
                                            stop=True,
                                        )
                                    nc.scalar.activation(
                                        out=e_t[:, half * 1024 : (half + 1) * 1024],
                                        in_=ps,
                                        func=AF.Exp,
                                        accum_out=racc[:, half : half + 1],
                                    )
                                rr = small.tile([128, 1], F32, tag="rr")
                                nc.vector.tensor_add(
                                    out=rr, in0=racc[:, 0:1], in1=racc[:, 1:2]
                                )
                                nc.vector.reciprocal(out=rr, in_=rr)
                                nc.sync.dma_start(
                                    out=rr_t[h, qb * 128 : (qb + 1) * 128], in_=rr
                                )
                                nc.gpsimd.tensor_scalar_mul(
                                    out=e_t, in0=e_t, scalar1=rr
                                )
                                nc.sync.dma_start(
                                    out=attn_d[h, qb * 128 : (qb + 1) * 128, :],
                                    in_=e_t,
                                )
                        # pass B: transposed scores -> context
                        for h in heads:
                            pb, mt = (h % 2) * 64, h // 2
                            rrb = rrbp.tile([64, SQ], F32, tag="rrb")
                            nc.sync.dma_start(
                                out=rrb, in_=_bcast(rr_t[h, :], 64)
                            )
                            ctx = psC.tile([64, 512], F32, tag="ctx")
                            for kt in range(16):
                                psb = psB.tile([128, 512], F32, tag="sB")
                                nc.tensor.matmul(
                                    psb,
                                    KT[pb : pb + 64, mt, kt * 128 : (kt + 1) * 128],
                                    QT[pb : pb + 64, mt, :],
                                    start=True,
                                    stop=True,
                                )
                                et = etpool.tile([128, 512], F32R, tag="ET")
                                nc.scalar.activation(out=et, in_=psb, func=AF.Exp)
                                nc.tensor.matmul(
                                    ctx,
                                    Vn[:, kt, h * 64 : (h + 1) * 64],
                                    et,
                                    start=(kt == 0),
                                    stop=(kt == 15),
                                )
                            nc.vector.tensor_tensor(
                                out=CT[:, h, :],
                                in0=ctx,
                                in1=rrb,
                                op=OP.mult,
                            )

            # ---- output projection + residual + LayerNorm ----
            with (
                tc.tile_pool(name="epi", bufs=1) as ep,
                tc.tile_pool(name="ypool", bufs=2) as yp,
                tc.tile_pool(name="psO", bufs=2, space="PSUM") as psO,
            ):
                # WoT in [64, head, dout] layout to match CT's 64-partition rows
                WoT = ep.tile([64, H, D], F32R, tag="WoT")
                nc.sync.dma_start(
                    out=WoT, in_=woT_d.rearrange("(hh p) o -> p hh o", p=64)
                )
                qn = ep.tile([128, 4, D], F32, tag="qn")
                nc.sync.dma_start(
                    out=qn, in_=qN_d.rearrange("(qs p) d -> p qs d", p=128)
                )
                badd_b = ep.tile([128, D], F32, tag="badd_b")
                nc.sync.dma_start(out=badd_b, in_=_bcast(badd_d[:], 128))
                gam_b = ep.tile([128, D], F32, tag="gam_b")
                nc.sync.dma_start(out=gam_b, in_=_bcast(gam_d[:], 128))
                bet_b = ep.tile([128, D], F32, tag="bet_b")
                nc.sync.dma_start(out=bet_b, in_=_bcast(bet_d[:], 128))
                eps_t = small.tile([128, 1], F32, tag="eps")
                nc.vector.memset(eps_t, LN_EPS)

                for qs in range(4):
                    y = yp.tile([128, D], F32, tag="y")
                    for nh in range(2):
                        ps = psO.tile([128, 512], F32, tag="po")
                        for hh in range(H):
                            nc.tensor.matmul(
                                ps,
                                CT[:, hh, qs * 128 : (qs + 1) * 128],
                                WoT[:, hh, nh * 512 : (nh + 1) * 512],
                                start=(hh == 0),
                                stop=(hh == H - 1),
                            )
                        sl = slice(nh * 512, (nh + 1) * 512)
                        nc.vector.tensor_add(
                            out=y[:, sl], in0=ps, in1=qn[:, qs, sl]
                        )
                        nc.vector.tensor_add(
                            out=y[:, sl], in0=y[:, sl], in1=badd_b[:, sl]
                        )
                    stats = small.tile([128, 2, 6], F32, tag="stats")
                    for sg in range(2):
                        nc.vector.bn_stats(
                            out=stats[:, sg, :], in_=y[:, sg * 512 : (sg + 1) * 512]
                        )
                    mv = small.tile([128, 2], F32, tag="mv")
                    nc.vector.bn_aggr(out=mv, in_=stats)
                    rstd = small.tile([128, 1], F32, tag="rstd")
                    nc.scalar.activation(
                        out=rstd,
                        in_=mv[:, 1:2],
                        func=AF.Sqrt,
                        bias=eps_t,
                        scale=1.0,
                    )
                    nc.vector.reciprocal(out=rstd, in_=rstd)
                    t = yp.tile([128, D], F32, tag="t")
                    nc.vector.scalar_tensor_tensor(
                        out=t,
                        in0=y,
                        scalar=mv[:, 0:1],
                        in1=gam_b,
                        op0=OP.subtract,
                        op1=OP.mult,
                    )
                    nc.vector.tensor_scalar_mul(out=t, in0=t, scalar1=rstd)
                    nc.vector.tensor_tensor(out=t, in0=t, in1=bet_b, op=OP.add)
                    nc.sync.dma_start(
                        out=out_d[qs * 128 : (qs + 1) * 128, :], in_=t
                    )

    nc.compile()
    return nc


def _get_nc():
    global _NC_CACHE
    if _NC_CACHE is None:
        _NC_CACHE = build()
    return _NC_CACHE


def kernel(query, key, value, Wq, bq, Wk, bk, Wv, bv, Wo, bo, gamma, beta):
    global LAST_EXEC_NS
    f32 = lambda x: np.asarray(x, dtype=np.float32)
    query, key, value = f32(query), f32(key), f32(value)
    Wq, bq, Wk, bk = f32(Wq), f32(bq), f32(Wk), f32(bk)
    Wv, bv, Wo, bo = f32(Wv), f32(bv), f32(Wo), f32(bo)
    gamma, beta = f32(gamma), f32(beta)

    scale = np.float32(1.0 / np.sqrt(np.float32(DH)))  # 0.125, exact
    wqT = np.ascontiguousarray(Wq.T) * scale
    bq_s = bq * scale
    wkT = np.ascontiguousarray(Wk.T)
    wvT = np.ascontiguousarray(Wv.T)
    woT = np.ascontiguousarray(Wo.T)
    badd = (
        bo.astype(np.float64) + Wo.astype(np.float64) @ bv.astype(np.float64)
    ).astype(np.float32)

    keyT = [np.ascontiguousarray(key[b].T) for b in range(B)]
    valueT = [np.ascontiguousarray(value[b].T) for b in range(B)]

    in_maps = []
    for c in range(NCORES):
        b, qs = c // 4, c % 4
        sl = slice(qs * SQ, (qs + 1) * SQ)
        in_maps.append(
            {
                "queryT": np.ascontiguousarray(query[b, sl, :].T),
                "query_nat": np.ascontiguousarray(query[b, sl, :]),
                "keyT": keyT[b],
                "valueT": valueT[b],
                "wqT": wqT,
                "wkT": wkT,
                "wvT": wvT,
                "woT": woT,
                "bq": bq_s,
                "bk": bk,
                "badd": badd,
                "gamma": gamma,
                "beta": beta,
            }
        )

    res = run_bass_kernel_spmd(
        _get_nc(),
        in_maps,
        list(range(NCORES)),
        trace=PROFILE,
        tmpdir=TRACE_DIR,
    )
    LAST_EXEC_NS = res.exec_time_ns

    output = np.empty((B, S, D), np.float32)
    attn = np.empty((H * B, S, S), np.float32)
    for c in range(NCORES):
        b, qs = c // 4, c % 4
        sl = slice(qs * SQ, (qs + 1) * SQ)
        output[b, sl, :] = res.results[c]["out"]
        a = res.results[c]["attn"]
        for h in range(H):
            attn[h * B + b, sl, :] = a[h]
    return output, attn
